# revision 15
# baseline (speedup 1.0000x reference)
"""Multi-head causal attention (bs=4, L=2048, d_model=512, 8 heads x 64) on 8
Trainium2 NeuronCores.

Sharding: core c = (batch b = c//2, head-group hg = c%2); each core computes 4
heads of one batch over the full sequence. Host pre-transposes activations and
weight slices so every device matmul has its contraction dim on partitions;
device returns the transposed partial output projection; host sums the two
head-group partials per batch, transposes back and adds the (folded) biases.

Schedule: the attention inner loop is kept ACT-subordinate (one N=2048 exp per
2-j-tile group covering both head pairs) while deferred projection chunks are
drip-fed into the PE stream as filler so the tensor engine never micro-idles
(HAM re-throttles the PE clock on idle windows).
"""

import numpy as np

import concourse.bacc as bacc
import concourse.mybir as mybir
import concourse.tile as tile
from concourse.bass_utils import run_bass_kernel_spmd

F32 = mybir.dt.float32
F16 = mybir.dt.float16
AF = mybir.ActivationFunctionType

L = 2048          # sequence length
D = 512           # model dim
HD = 256          # head-group output dim (4 heads x 64)
DK = 64           # head dim
NH = 4            # heads per core
P = 128
IB = 512          # query block (i) width
NIB = L // IB     # 4 query blocks
NKT = D // P      # 4 contraction tiles over model dim
NJT = L // P      # 16 key tiles
SCALE = 1.0 / 8.0  # 1/sqrt(DK)

GRP = 1           # score j-tiles per PSUM/exp group (x2 head pairs)


def _build():
    nc = bacc.Bacc("TRN2", target_bir_lowering=False, debug=False,
                   enable_asserts=False)

    xT = nc.dram_tensor("xT", [D, L], F16, kind="ExternalInput")
    wq = nc.dram_tensor("wq", [P, NKT, HD], F16, kind="ExternalInput")
    wk = nc.dram_tensor("wk", [P, NKT, HD], F16, kind="ExternalInput")
    wv = nc.dram_tensor("wv", [P, NKT, HD], F16, kind="ExternalInput")
    wo = nc.dram_tensor("wo", [P, HD // P, D], F16, kind="ExternalInput")
    bqk = nc.dram_tensor("bqk", [P, 4], F32, kind="ExternalInput")
    outT = nc.dram_tensor("outT", [D, L], F32, kind="ExternalOutput")

    with tile.TileContext(nc) as tc:
        with (
            tc.tile_pool(name="w", bufs=1) as pool_w,
            tc.tile_pool(name="x", bufs=NKT) as pool_x,
            tc.tile_pool(name="qk", bufs=1) as pool_qk,
            tc.tile_pool(name="v", bufs=NJT) as pool_v,
            tc.tile_pool(name="at", bufs=4) as pool_at,
            tc.tile_pool(name="zc", bufs=2) as pool_zc,
            tc.tile_pool(name="nm", bufs=2) as pool_nm,
            tc.tile_pool(name="o", bufs=2) as pool_o,
            tc.tile_pool(name="ps", bufs=2, space="PSUM") as pool_ps,
            tc.tile_pool(name="pz", bufs=2, space="PSUM") as pool_pz,
            tc.tile_pool(name="pp", bufs=2, space="PSUM") as pool_pp,
        ):
            # ---- loads: everything host-side is pre-laid-out so each DMA is
            # one contiguous run per partition (128 fat descriptors, not 512
            # scattered ones). Order: what the first matmuls need first; the
            # back halves of xT and wo arrive while attention is running.
            # PE warmup: dummy matmuls on garbage SBUF keep the PE busy
            # through the HAM SHORT window while the input DMAs run, so the
            # real matmuls start at 2.4 GHz instead of 1.2.
            wrm = pool_w.tile([P, IB], F16, tag="wrm")
            nc.vector.memset(wrm[:], 0.0)
            # dummy PartitionBroadcast: the first use of this op makes gpsimd
            # fetch a new ucode library (~7us!); pay that cost here, overlapped
            # with the load prologue, instead of inside the first normalize.
            wbc = pool_w.tile([P, IB], F32, tag="wbc")
            nc.vector.memset(wbc[0:1, :], 1.0)
            nc.gpsimd.partition_broadcast(wbc[DK:2 * DK, :], wbc[0:1, :],
                                          channels=DK)
            for w_i in range(20):
                pw = pool_pp.tile([P, IB], F32, tag="pp", name="pp")
                nc.tensor.matmul(pw[:], lhsT=wrm[:, 0:P], rhs=wrm[:],
                                 start=True, stop=True)

            bqk_sb = pool_w.tile([P, 4], F32, tag="bqk")
            wq_sb = pool_w.tile([P, NKT, HD], F16, tag="wq")
            wk_sb = pool_w.tile([P, NKT, HD], F16, tag="wk")
            wv_sb = pool_w.tile([P, NKT, HD], F16, tag="wv")
            wo_sb = pool_w.tile([P, HD // P, D], F16, tag="wo")
            nc.sync.dma_start(bqk_sb[:], bqk.ap())
            nc.sync.dma_start(wq_sb[:], wq.ap())
            nc.sync.dma_start(wk_sb[:], wk.ap())
            xts = []
            for kt in range(NKT):
                xt = pool_x.tile([P, L], F16)
                nc.sync.dma_start(xt[:, 0:L // 2],
                                  xT.ap()[kt * P:(kt + 1) * P, 0:L // 2])
                xts.append(xt)
            nc.sync.dma_start(wv_sb[:], wv.ap())
            for kt in range(NKT):
                nc.sync.dma_start(xts[kt][:, L // 2:L],
                                  xT.ap()[kt * P:(kt + 1) * P, L // 2:L])
            nc.sync.dma_start(wo_sb[:], wo.ap())

            # ---- q/k projections, one (dt, ic) chunk at a time ----
            qk_tiles = {}
            for name in ("q", "k"):
                for dt in range(2):
                    qk_tiles[(name, dt)] = pool_qk.tile(
                        [P, L], F16, tag=f"{name}{dt}", name=f"{name}{dt}")

            def emit_qk_chunk(name, dt, ic):
                w_sb = wq_sb if name == "q" else wk_sb
                bcol = dt if name == "q" else 2 + dt
                dst = qk_tiles[(name, dt)]
                pp = pool_pp.tile([P, IB], F32, tag="pp", name="pp")
                for kt in range(NKT):
                    nc.tensor.matmul(
                        pp[:],
                        lhsT=w_sb[:, kt, dt * P:(dt + 1) * P],
                        rhs=xts[kt][:, ic * IB:(ic + 1) * IB],
                        start=(kt == 0), stop=(kt == NKT - 1),
                    )
                nc.vector.tensor_scalar_add(
                    dst[:, ic * IB:(ic + 1) * IB], pp[:],
                    bqk_sb[:, bcol:bcol + 1])

            # ---- v projection: natural layout [j, (h, 65)], col 64 == 1.0 ----
            vts = [None] * NJT

            def emit_v(jt):
                vt = pool_v.tile([P, NH, DK + 1], F16, tag="v", name="v")
                pp = pool_pp.tile([P, HD], F32, tag="pp", name="pp")
                for kt in range(NKT):
                    nc.tensor.matmul(
                        pp[:],
                        lhsT=xts[kt][:, jt * P:(jt + 1) * P],
                        rhs=wv_sb[:, kt, :],
                        start=(kt == 0), stop=(kt == NKT - 1),
                    )
                nc.vector.tensor_copy(
                    vt[:, :, 0:DK],
                    pp[:].rearrange("p (h e) -> p h e", h=NH))
                nc.gpsimd.memset(vt[:, :, DK:DK + 1], 1.0)
                vts[jt] = vt

            def outproj_mt(ib, mt):
                zc = zcs[ib]
                po = pool_pp.tile([P, IB], F32, tag="pp", name="pp")
                for kt2 in range(HD // P):
                    nc.tensor.matmul(
                        po[:],
                        lhsT=wo_sb[:, kt2, mt * P:(mt + 1) * P],
                        rhs=zc[kt2][:],
                        start=(kt2 == 0), stop=(kt2 == HD // P - 1),
                    )
                osb = pool_o.tile([P, IB], F32, tag="o", name="o")
                if ib == NIB - 1:
                    for h in range(2):
                        cs = slice(h * (IB // 2), (h + 1) * (IB // 2))
                        nc.scalar.copy(osb[:, cs], po[:, cs])
                        nc.sync.dma_start(
                            outT.ap()[mt * P:(mt + 1) * P,
                                      ib * IB + h * (IB // 2):
                                      ib * IB + (h + 1) * (IB // 2)],
                            osb[:, cs])
                else:
                    nc.vector.tensor_copy(osb[:], po[:])
                    nc.sync.dma_start(
                        outT.ap()[mt * P:(mt + 1) * P,
                                  ib * IB:(ib + 1) * IB],
                        osb[:])


            # ---- deferred projection work, statically woven into the PE
            # stream: PLAN[(attn-idx, j-tile)] = closures emitted right after
            # that j-tile's score/z matmuls. Placement rules: a qk chunk
            # (dt, ic) lands before the attention pair that reads it, v j-tiles
            # land before the query block whose z consumes them, and each
            # outproj(ib) rides a couple of j-tiles into attn(ib+1, 0) so its
            # matmuls never head-of-line-block on the normalize chain.
            def E(name, dt, ic):
                return lambda: emit_qk_chunk(name, dt, ic)

            def V(jt):
                return lambda: emit_v(jt)

            def O(ib, mt):
                return lambda: outproj_mt(ib, mt)

            PLAN = {
                (0, 1): [E("q", 1, 0)], (0, 2): [E("k", 1, 0)],
                (0, 3): [V(4)],
                (1, 0): [V(5)], (1, 1): [V(6)], (1, 2): [V(7)],
                (1, 3): [E("q", 0, 1)],
                (2, 0): [E("k", 0, 1)], (2, 1): [E("q", 1, 1)],
                (2, 2): [O(0, 0)], (2, 3): [O(0, 1)], (2, 4): [O(0, 2)],
                (2, 5): [O(0, 3)], (2, 6): [E("k", 1, 1)], (2, 7): [V(8)],
                (3, 0): [V(9)], (3, 1): [V(10)], (3, 2): [V(11)],
                (3, 3): [E("q", 0, 2)], (3, 4): [E("k", 0, 2)],
                (4, 2): [O(1, 0)], (4, 3): [O(1, 1)], (4, 4): [O(1, 2)],
                (4, 5): [O(1, 3)], (4, 6): [E("q", 1, 2)],
                (4, 7): [E("k", 1, 2)], (4, 8): [V(12)], (4, 9): [V(13)],
                (5, 0): [V(14)], (5, 1): [V(15)], (5, 2): [E("q", 0, 3)],
                (5, 3): [E("k", 0, 3)],
                (6, 2): [O(2, 0)], (6, 3): [O(2, 1)],
                (6, 6): [E("q", 1, 3)], (6, 7): [E("k", 1, 3)],
                (7, 2): [O(2, 2)], (7, 3): [O(2, 3)],
            }

            def pop_filler(now_idx, jb):
                for fn in PLAN.pop((now_idx, jb), []):
                    fn()

            # ---- attention per (query block, head pair) ----
            zcs = {}

            def new_zc(ib):
                zcs[ib] = [pool_zc.tile([P, IB], F16, tag=f"zc{dt}",
                                        name=f"zc{dt}")
                           for dt in range(HD // P)]

            def attn_pair(ib, hp):
                idx = 2 * ib + hp
                zc = zcs[ib]
                qt = qk_tiles[("q", hp)]
                kt_t = qk_tiles[("k", hp)]
                nj = 4 * (ib + 1)
                pszs = [pool_pz.tile([P, IB], F32, tag="pz", name=f"pz{par}")
                        for par in range(2)]

                def emit_z(jb, z_at):
                    zc0 = P * max(jb - 4 * ib, 0)
                    for par in range(2):
                        h = 2 * hp + par
                        nc.tensor.matmul(
                            pszs[par][0:DK + 1, zc0:IB],
                            lhsT=vts[jb][:, h, :],
                            rhs=z_at[:, par, zc0:IB],
                            start=(jb == 0), stop=(jb == nj - 1),
                        )

                prev = None  # z matmuls lag the scores by one j-tile so
                # the exp+mask latency never stalls the in-order PE stream
                for jb in range(nj):
                    # diagonal j-tile u: queries c < 128u are causally dead --
                    # skip them in the scores matmul, the exp, and the z
                    # matmul; memset the dead at region; triangular select on
                    # the [128,128] block at c-128u.
                    u = jb - 4 * ib
                    c0 = P * max(u, 0)  # first live query column
                    ps = pool_ps.tile([P, 2, IB], F32, tag="ps", name="ps")
                    at = pool_at.tile([P, 2, IB], F16, tag="at", name="at")
                    for par in range(2):
                        drow = DK * par
                        nc.tensor.matmul(
                            ps[:, par, c0:IB],
                            lhsT=kt_t[drow:drow + DK,
                                      jb * P:(jb + 1) * P],
                            rhs=qt[drow:drow + DK,
                                   ib * IB + c0:(ib + 1) * IB],
                            start=True, stop=True,
                        )
                    if prev is not None:
                        emit_z(*prev)
                    nc.scalar.activation(at[:, :, c0:IB], ps[:, :, c0:IB],
                                         AF.Exp, scale=SCALE)
                    if u >= 0:
                        for par in range(2):
                            at_s = at[:, par, :]
                            if u > 0:
                                nc.gpsimd.memset(at_s[0:P, 0:c0], 0.0)
                            nc.gpsimd.affine_select(
                                at_s[0:P, c0:c0 + P],
                                at_s[0:P, c0:c0 + P],
                                pattern=[[1, P]],
                                compare_op=mybir.AluOpType.is_ge,
                                fill=0.0, base=0,
                                channel_multiplier=-1,
                            )
                    pop_filler(idx, jb)
                    prev = (jb, at)
                emit_z(*prev)
                # normalize: z / denom (den row = psum partition 64). par 1
                # first so its partition-shift DMA hop overlaps par 0's chain.
                for par in (1, 0):
                    psz = pszs[par]
                    if ib == NIB - 1:
                        # tail: nothing competes for the PSUM bank; skip the
                        # evacuation and normalize straight out of PSUM
                        den = pool_nm.tile([P, IB], F32, tag="den",
                                           name="den")
                        bct = pool_nm.tile([P, IB], F32, tag="bct",
                                           name="bct")
                        nc.vector.tensor_copy(den[DK:DK + 1, :],
                                              psz[DK:DK + 1, :])
                        nc.gpsimd.dma_start(den[0:1, :], den[DK:DK + 1, :])
                        nc.vector.reciprocal_approx_fast(den[0:1, :],
                                                         den[0:1, :])
                        nc.gpsimd.partition_broadcast(
                            bct[0:DK, :], den[0:1, :], channels=DK)
                        if par == 0:
                            nc.vector.tensor_mul(zc[hp][0:DK, :],
                                                 psz[0:DK, :], bct[0:DK, :])
                        else:
                            zn = pool_nm.tile([P, IB], F16, tag="zn",
                                              name="zn")
                            nc.vector.tensor_mul(zn[0:DK, :],
                                                 psz[0:DK, :], bct[0:DK, :])
                            nc.gpsimd.dma_start(zc[hp][DK:P, :], zn[0:DK, :])
                        continue
                    zsb = pool_nm.tile([P, IB], F32, tag="zsb", name="zsb")
                    den = pool_nm.tile([P, IB], F32, tag="den", name="den")
                    bct = pool_nm.tile([P, IB], F32, tag="bct", name="bct")
                    # one copy evacuates z+den so the PSUM bank frees for the
                    # next head pair immediately
                    nc.vector.tensor_copy(zsb[0:DK + 1, :],
                                          psz[0:DK + 1, :])
                    nc.sync.dma_start(den[0:1, :], zsb[DK:DK + 1, :])
                    nc.vector.reciprocal_approx_fast(den[0:1, :],
                                                     den[0:1, :])
                    nc.gpsimd.partition_broadcast(
                        bct[0:DK, :], den[0:1, :], channels=DK)
                    if par == 0:
                        nc.vector.tensor_mul(zc[hp][0:DK, :],
                                             zsb[0:DK, :], bct[0:DK, :])
                    else:
                        # DVE lanes are partition-locked; shift the odd
                        # head's rows 0:64 -> 64:128 via an SBUF DMA hop
                        zn = pool_nm.tile([P, IB], F16, tag="zn", name="zn")
                        nc.vector.tensor_mul(zn[0:DK, :],
                                             zsb[0:DK, :], bct[0:DK, :])
                        nc.sync.dma_start(zc[hp][DK:P, :], zn[0:DK, :])

            # ---- emission schedule ----
            emit_qk_chunk("q", 0, 0)
            emit_qk_chunk("k", 0, 0)
            for jt in range(4):
                emit_v(jt)
            for ib in range(NIB):
                new_zc(ib)
                for hp in range(2):
                    attn_pair(ib, hp)
            # keep the PE busy (and the HAM clock warm) while the last
            # normalize chain drains, so the final output projection runs at
            # full clock. rhs reads zc(3)[0] purely to pin these at the tail:
            # with no deps the scheduler would hoist them into earlier gaps.
            for w_i in range(24):
                pw = pool_pp.tile([P, IB], F32, tag="pp", name="pp")
                nc.tensor.matmul(pw[:], lhsT=wrm[:, 0:P],
                                 rhs=zcs[NIB - 1][0][:],
                                 start=True, stop=True)
            for mt in range(D // P):
                outproj_mt(NIB - 1, mt)

    nc.compile()
    return nc


_NC = None


def _get_nc():
    global _NC
    if _NC is None:
        _NC = _build()
    return _NC


def _in_maps(x, w_q, b_q, w_k, b_k, w_v, b_v, w_o, b_o):
    def wlay(w):  # [HD, D] slice -> [P, NKT, HD] (partition-contiguous)
        return np.ascontiguousarray(
            w.T.astype(np.float16).reshape(NKT, P, HD).transpose(1, 0, 2))

    maps = []
    for b in range(4):
        xTb = np.ascontiguousarray(x[b].T.astype(np.float32)).astype(np.float16)
        for hg in range(2):
            sl = slice(hg * HD, (hg + 1) * HD)
            bqk = np.stack([
                b_q[sl][0:P], b_q[sl][P:HD], b_k[sl][0:P], b_k[sl][P:HD],
            ], axis=1).astype(np.float32)
            maps.append({
                "xT": xTb,
                "wq": wlay(w_q[sl]),
                "wk": wlay(w_k[sl]),
                "wv": wlay(w_v[sl]),
                "wo": np.ascontiguousarray(
                    w_o[:, sl].T.astype(np.float16).reshape(
                        HD // P, P, D).transpose(1, 0, 2)),
                "bqk": np.ascontiguousarray(bqk),
            })
    return maps


def _combine(results, w_o, b_v, b_o):
    corr = (b_o + w_o @ b_v).astype(np.float32)  # fold v/out biases
    out = np.empty((4, L, D), dtype=np.float32)
    for b in range(4):
        acc = results[2 * b]["outT"] + results[2 * b + 1]["outT"]
        out[b] = acc.T + corr
    return out


def kernel(x, w_q, b_q, w_k, b_k, w_v, b_v, w_o, b_o):
    nc = _get_nc()
    maps = _in_maps(x, w_q, b_q, w_k, b_k, w_v, b_v, w_o, b_o)
    res = run_bass_kernel_spmd(nc, maps, core_ids=list(range(8)))
    return _combine(res.results, w_o, b_v, b_o)


def bench(x, w_q, b_q, w_k, b_k, w_v, b_v, w_o, b_o):
    """Run with NTFF tracing; returns (output, exec_time_ns)."""
    nc = _get_nc()
    maps = _in_maps(x, w_q, b_q, w_k, b_k, w_v, b_v, w_o, b_o)
    res = run_bass_kernel_spmd(nc, maps, core_ids=list(range(8)), trace=True)
    return _combine(res.results, w_o, b_v, b_o), res.exec_time_ns


# revision 16
# speedup vs baseline: 1.0205x; 1.0205x over previous
"""Multi-head causal attention (bs=4, L=2048, d_model=512, 8 heads x 64) on 8
Trainium2 NeuronCores.

Sharding: core c = (batch b = c//2, head-group hg = c%2); each core computes 4
heads of one batch over the full sequence. Host pre-transposes activations and
weight slices so every device matmul has its contraction dim on partitions;
device returns the transposed partial output projection; host sums the two
head-group partials per batch, transposes back and adds the (folded) biases.

Schedule: the attention inner loop is kept ACT-subordinate (one N=2048 exp per
2-j-tile group covering both head pairs) while deferred projection chunks are
drip-fed into the PE stream as filler so the tensor engine never micro-idles
(HAM re-throttles the PE clock on idle windows).
"""

import numpy as np

import concourse.bacc as bacc
import concourse.mybir as mybir
import concourse.tile as tile
from concourse.bass_utils import run_bass_kernel_spmd

F32 = mybir.dt.float32
F16 = mybir.dt.float16
AF = mybir.ActivationFunctionType

L = 2048          # sequence length
D = 512           # model dim
HD = 256          # head-group output dim (4 heads x 64)
DK = 64           # head dim
NH = 4            # heads per core
P = 128
IB = 512          # query block (i) width
NIB = L // IB     # 4 query blocks
NKT = D // P      # 4 contraction tiles over model dim
NJT = L // P      # 16 key tiles
SCALE = 1.0 / 8.0  # 1/sqrt(DK)

GRP = 1           # score j-tiles per PSUM/exp group (x2 head pairs)


def _build():
    nc = bacc.Bacc("TRN2", target_bir_lowering=False, debug=False,
                   enable_asserts=False)

    xT = nc.dram_tensor("xT", [D, L], F16, kind="ExternalInput")
    wq = nc.dram_tensor("wq", [P, NKT, HD], F16, kind="ExternalInput")
    wk = nc.dram_tensor("wk", [P, NKT, HD], F16, kind="ExternalInput")
    wv = nc.dram_tensor("wv", [P, NKT, HD], F16, kind="ExternalInput")
    wo = nc.dram_tensor("wo", [P, HD // P, D], F16, kind="ExternalInput")
    bqk = nc.dram_tensor("bqk", [P, 4], F32, kind="ExternalInput")
    outT = nc.dram_tensor("outT", [D, L], F32, kind="ExternalOutput")

    with tile.TileContext(nc) as tc:
        with (
            tc.tile_pool(name="w", bufs=1) as pool_w,
            tc.tile_pool(name="x", bufs=NKT) as pool_x,
            tc.tile_pool(name="qk", bufs=1) as pool_qk,
            tc.tile_pool(name="v", bufs=NJT) as pool_v,
            tc.tile_pool(name="at", bufs=4) as pool_at,
            tc.tile_pool(name="zc", bufs=2) as pool_zc,
            tc.tile_pool(name="nm", bufs=2) as pool_nm,
            tc.tile_pool(name="o", bufs=2) as pool_o,
            tc.tile_pool(name="ps", bufs=2, space="PSUM") as pool_ps,
            tc.tile_pool(name="pz", bufs=2, space="PSUM") as pool_pz,
            tc.tile_pool(name="pp", bufs=2, space="PSUM") as pool_pp,
        ):
            # ---- loads: everything host-side is pre-laid-out so each DMA is
            # one contiguous run per partition (128 fat descriptors, not 512
            # scattered ones). Order: what the first matmuls need first; the
            # back halves of xT and wo arrive while attention is running.
            # PE warmup: dummy matmuls on garbage SBUF keep the PE busy
            # through the HAM SHORT window while the input DMAs run, so the
            # real matmuls start at 2.4 GHz instead of 1.2.
            wrm = pool_w.tile([P, IB], F16, tag="wrm")
            nc.vector.memset(wrm[:], 0.0)
            # dummy PartitionBroadcast: the first use of this op makes gpsimd
            # fetch a new ucode library (~7us!); pay that cost here, overlapped
            # with the load prologue, instead of inside the first normalize.
            wbc = pool_w.tile([P, IB], F32, tag="wbc")
            nc.vector.memset(wbc[0:1, :], 1.0)
            nc.gpsimd.partition_broadcast(wbc[DK:2 * DK, :], wbc[0:1, :],
                                          channels=DK)
            for w_i in range(20):
                pw = pool_pp.tile([P, IB], F32, tag="pp", name="pp")
                nc.tensor.matmul(pw[:], lhsT=wrm[:, 0:P], rhs=wrm[:],
                                 start=True, stop=True)

            bqk_sb = pool_w.tile([P, 4], F32, tag="bqk")
            wq_sb = pool_w.tile([P, NKT, HD], F16, tag="wq")
            wk_sb = pool_w.tile([P, NKT, HD], F16, tag="wk")
            wv_sb = pool_w.tile([P, NKT, HD], F16, tag="wv")
            wo_sb = pool_w.tile([P, HD // P, D], F16, tag="wo")
            nc.sync.dma_start(bqk_sb[:], bqk.ap())
            nc.sync.dma_start(wq_sb[:], wq.ap())
            nc.sync.dma_start(wk_sb[:], wk.ap())
            xts = []
            for kt in range(NKT):
                xt = pool_x.tile([P, L], F16)
                nc.sync.dma_start(xt[:, 0:L // 2],
                                  xT.ap()[kt * P:(kt + 1) * P, 0:L // 2])
                xts.append(xt)
            nc.sync.dma_start(wv_sb[:], wv.ap())
            for kt in range(NKT):
                nc.sync.dma_start(xts[kt][:, L // 2:L],
                                  xT.ap()[kt * P:(kt + 1) * P, L // 2:L])
            nc.sync.dma_start(wo_sb[:], wo.ap())

            # ---- q/k projections, one (dt, ic) chunk at a time ----
            qk_tiles = {}
            for name in ("q", "k"):
                for dt in range(2):
                    qk_tiles[(name, dt)] = pool_qk.tile(
                        [P, L], F16, tag=f"{name}{dt}", name=f"{name}{dt}")

            def emit_qk_chunk(name, dt, ic):
                w_sb = wq_sb if name == "q" else wk_sb
                bcol = dt if name == "q" else 2 + dt
                dst = qk_tiles[(name, dt)]
                pp = pool_pp.tile([P, IB], F32, tag="pp", name="pp")
                for kt in range(NKT):
                    nc.tensor.matmul(
                        pp[:],
                        lhsT=w_sb[:, kt, dt * P:(dt + 1) * P],
                        rhs=xts[kt][:, ic * IB:(ic + 1) * IB],
                        start=(kt == 0), stop=(kt == NKT - 1),
                    )
                nc.vector.tensor_scalar_add(
                    dst[:, ic * IB:(ic + 1) * IB], pp[:],
                    bqk_sb[:, bcol:bcol + 1])

            # ---- v projection: natural layout [j, (h, 65)], col 64 == 1.0 ----
            vts = [None] * NJT

            def emit_v(jt):
                vt = pool_v.tile([P, NH, DK + 1], F16, tag="v", name="v")
                pp = pool_pp.tile([P, HD], F32, tag="pp", name="pp")
                for kt in range(NKT):
                    nc.tensor.matmul(
                        pp[:],
                        lhsT=xts[kt][:, jt * P:(jt + 1) * P],
                        rhs=wv_sb[:, kt, :],
                        start=(kt == 0), stop=(kt == NKT - 1),
                    )
                nc.vector.tensor_copy(
                    vt[:, :, 0:DK],
                    pp[:].rearrange("p (h e) -> p h e", h=NH))
                nc.gpsimd.memset(vt[:, :, DK:DK + 1], 1.0)
                vts[jt] = vt

            def outproj_mt(ib, mt):
                zc = zcs[ib]
                po = pool_pp.tile([P, IB], F32, tag="pp", name="pp")
                for kt2 in range(HD // P):
                    nc.tensor.matmul(
                        po[:],
                        lhsT=wo_sb[:, kt2, mt * P:(mt + 1) * P],
                        rhs=zc[kt2][:],
                        start=(kt2 == 0), stop=(kt2 == HD // P - 1),
                    )
                osb = pool_o.tile([P, IB], F32, tag="o", name="o")
                if ib == NIB - 1:
                    nc.scalar.copy(osb[:], po[:])
                else:
                    nc.vector.tensor_copy(osb[:], po[:])
                nc.sync.dma_start(
                    outT.ap()[mt * P:(mt + 1) * P, ib * IB:(ib + 1) * IB],
                    osb[:])


            # ---- deferred projection work, statically woven into the PE
            # stream: PLAN[(attn-idx, j-tile)] = closures emitted right after
            # that j-tile's score/z matmuls. Placement rules: a qk chunk
            # (dt, ic) lands before the attention pair that reads it, v j-tiles
            # land before the query block whose z consumes them, and each
            # outproj(ib) rides a couple of j-tiles into attn(ib+1, 0) so its
            # matmuls never head-of-line-block on the normalize chain.
            def E(name, dt, ic):
                return lambda: emit_qk_chunk(name, dt, ic)

            def V(jt):
                return lambda: emit_v(jt)

            def O(ib, mt):
                return lambda: outproj_mt(ib, mt)

            PLAN = {
                (0, 1): [E("q", 1, 0)], (0, 2): [E("k", 1, 0)],
                (0, 3): [V(4)],
                (1, 0): [V(5)], (1, 1): [V(6)], (1, 2): [V(7)],
                (1, 3): [E("q", 0, 1)],
                (2, 0): [E("k", 0, 1)], (2, 1): [E("q", 1, 1)],
                (2, 2): [O(0, 0)], (2, 3): [O(0, 1)], (2, 4): [O(0, 2)],
                (2, 5): [O(0, 3)], (2, 6): [E("k", 1, 1)], (2, 7): [V(8)],
                (3, 0): [V(9)], (3, 1): [V(10)], (3, 2): [V(11)],
                (3, 3): [E("q", 0, 2)], (3, 4): [E("k", 0, 2)],
                (4, 2): [O(1, 0)], (4, 3): [O(1, 1)], (4, 4): [O(1, 2)],
                (4, 5): [O(1, 3)], (4, 6): [E("q", 1, 2)],
                (4, 7): [E("k", 1, 2)], (4, 8): [V(12)], (4, 9): [V(13)],
                (5, 0): [V(14)], (5, 1): [V(15)], (5, 2): [E("q", 0, 3)],
                (5, 3): [E("k", 0, 3)],
                (6, 2): [O(2, 0)], (6, 3): [O(2, 1)],
                (6, 6): [E("q", 1, 3)], (6, 7): [E("k", 1, 3)],
                (7, 2): [O(2, 2)], (7, 3): [O(2, 3)],
            }

            def pop_filler(now_idx, jb):
                for fn in PLAN.pop((now_idx, jb), []):
                    fn()

            # ---- attention per (query block, head pair) ----
            zcs = {}

            def new_zc(ib):
                zcs[ib] = [pool_zc.tile([P, IB], F16, tag=f"zc{dt}",
                                        name=f"zc{dt}")
                           for dt in range(HD // P)]

            def attn_pair(ib, hp):
                idx = 2 * ib + hp
                zc = zcs[ib]
                qt = qk_tiles[("q", hp)]
                kt_t = qk_tiles[("k", hp)]
                nj = 4 * (ib + 1)
                pszs = [pool_pz.tile([P, IB], F32, tag="pz", name=f"pz{par}")
                        for par in range(2)]

                def emit_z(jb, z_at):
                    zc0 = P * max(jb - 4 * ib, 0)
                    for par in range(2):
                        h = 2 * hp + par
                        nc.tensor.matmul(
                            pszs[par][0:DK + 1, zc0:IB],
                            lhsT=vts[jb][:, h, :],
                            rhs=z_at[:, par, zc0:IB],
                            start=(jb == 0), stop=(jb == nj - 1),
                        )

                prev = None  # z matmuls lag the scores by one j-tile so
                # the exp+mask latency never stalls the in-order PE stream
                for jb in range(nj):
                    # diagonal j-tile u: queries c < 128u are causally dead --
                    # skip them in the scores matmul, the exp, and the z
                    # matmul; memset the dead at region; triangular select on
                    # the [128,128] block at c-128u.
                    u = jb - 4 * ib
                    c0 = P * max(u, 0)  # first live query column
                    ps = pool_ps.tile([P, 2, IB], F32, tag="ps", name="ps")
                    at = pool_at.tile([P, 2, IB], F16, tag="at", name="at")
                    for par in range(2):
                        drow = DK * par
                        nc.tensor.matmul(
                            ps[:, par, c0:IB],
                            lhsT=kt_t[drow:drow + DK,
                                      jb * P:(jb + 1) * P],
                            rhs=qt[drow:drow + DK,
                                   ib * IB + c0:(ib + 1) * IB],
                            start=True, stop=True,
                        )
                    if prev is not None:
                        emit_z(*prev)
                    nc.scalar.activation(at[:, :, c0:IB], ps[:, :, c0:IB],
                                         AF.Exp, scale=SCALE)
                    if u >= 0:
                        for par in range(2):
                            at_s = at[:, par, :]
                            if u > 0:
                                nc.gpsimd.memset(at_s[0:P, 0:c0], 0.0)
                            nc.gpsimd.affine_select(
                                at_s[0:P, c0:c0 + P],
                                at_s[0:P, c0:c0 + P],
                                pattern=[[1, P]],
                                compare_op=mybir.AluOpType.is_ge,
                                fill=0.0, base=0,
                                channel_multiplier=-1,
                            )
                    pop_filler(idx, jb)
                    prev = (jb, at)
                emit_z(*prev)
                # normalize: z / denom (den row = psum partition 64). par 1
                # first so its partition-shift DMA hop overlaps par 0's chain.
                for par in (1, 0):
                    psz = pszs[par]
                    if ib == NIB - 1:
                        # tail: nothing competes for the PSUM bank; skip the
                        # evacuation and normalize straight out of PSUM
                        den = pool_nm.tile([P, IB], F32, tag="den",
                                           name="den")
                        bct = pool_nm.tile([P, IB], F32, tag="bct",
                                           name="bct")
                        nc.vector.tensor_copy(den[DK:DK + 1, :],
                                              psz[DK:DK + 1, :])
                        nc.sync.dma_start(den[0:1, :], den[DK:DK + 1, :])
                        nc.vector.reciprocal_approx_fast(den[0:1, :],
                                                         den[0:1, :])
                        nc.gpsimd.partition_broadcast(
                            bct[0:DK, :], den[0:1, :], channels=DK)
                        if par == 0:
                            nc.vector.tensor_mul(zc[hp][0:DK, :],
                                                 psz[0:DK, :], bct[0:DK, :])
                        else:
                            zn = pool_nm.tile([P, IB], F16, tag="zn",
                                              name="zn")
                            nc.vector.tensor_mul(zn[0:DK, :],
                                                 psz[0:DK, :], bct[0:DK, :])
                            nc.sync.dma_start(zc[hp][DK:P, :], zn[0:DK, :])
                        continue
                    zsb = pool_nm.tile([P, IB], F32, tag="zsb", name="zsb")
                    den = pool_nm.tile([P, IB], F32, tag="den", name="den")
                    bct = pool_nm.tile([P, IB], F32, tag="bct", name="bct")
                    # one copy evacuates z+den so the PSUM bank frees for the
                    # next head pair immediately
                    nc.vector.tensor_copy(zsb[0:DK + 1, :],
                                          psz[0:DK + 1, :])
                    nc.sync.dma_start(den[0:1, :], zsb[DK:DK + 1, :])
                    nc.vector.reciprocal_approx_fast(den[0:1, :],
                                                     den[0:1, :])
                    nc.gpsimd.partition_broadcast(
                        bct[0:DK, :], den[0:1, :], channels=DK)
                    if par == 0:
                        nc.vector.tensor_mul(zc[hp][0:DK, :],
                                             zsb[0:DK, :], bct[0:DK, :])
                    else:
                        # DVE lanes are partition-locked; shift the odd
                        # head's rows 0:64 -> 64:128 via an SBUF DMA hop
                        zn = pool_nm.tile([P, IB], F16, tag="zn", name="zn")
                        nc.vector.tensor_mul(zn[0:DK, :],
                                             zsb[0:DK, :], bct[0:DK, :])
                        nc.sync.dma_start(zc[hp][DK:P, :], zn[0:DK, :])

            # ---- emission schedule ----
            emit_qk_chunk("q", 0, 0)
            emit_qk_chunk("k", 0, 0)
            for jt in range(4):
                emit_v(jt)
            for ib in range(NIB):
                new_zc(ib)
                for hp in range(2):
                    attn_pair(ib, hp)
            for mt in range(D // P):
                outproj_mt(NIB - 1, mt)

    nc.compile()
    return nc


_NC = None


def _get_nc():
    global _NC
    if _NC is None:
        _NC = _build()
    return _NC


def _in_maps(x, w_q, b_q, w_k, b_k, w_v, b_v, w_o, b_o):
    def wlay(w):  # [HD, D] slice -> [P, NKT, HD] (partition-contiguous)
        return np.ascontiguousarray(
            w.T.astype(np.float16).reshape(NKT, P, HD).transpose(1, 0, 2))

    maps = []
    for b in range(4):
        xTb = np.ascontiguousarray(x[b].T.astype(np.float32)).astype(np.float16)
        for hg in range(2):
            sl = slice(hg * HD, (hg + 1) * HD)
            bqk = np.stack([
                b_q[sl][0:P], b_q[sl][P:HD], b_k[sl][0:P], b_k[sl][P:HD],
            ], axis=1).astype(np.float32)
            maps.append({
                "xT": xTb,
                "wq": wlay(w_q[sl]),
                "wk": wlay(w_k[sl]),
                "wv": wlay(w_v[sl]),
                "wo": np.ascontiguousarray(
                    w_o[:, sl].T.astype(np.float16).reshape(
                        HD // P, P, D).transpose(1, 0, 2)),
                "bqk": np.ascontiguousarray(bqk),
            })
    return maps


def _combine(results, w_o, b_v, b_o):
    corr = (b_o + w_o @ b_v).astype(np.float32)  # fold v/out biases
    out = np.empty((4, L, D), dtype=np.float32)
    for b in range(4):
        acc = results[2 * b]["outT"] + results[2 * b + 1]["outT"]
        out[b] = acc.T + corr
    return out


def kernel(x, w_q, b_q, w_k, b_k, w_v, b_v, w_o, b_o):
    nc = _get_nc()
    maps = _in_maps(x, w_q, b_q, w_k, b_k, w_v, b_v, w_o, b_o)
    res = run_bass_kernel_spmd(nc, maps, core_ids=list(range(8)))
    return _combine(res.results, w_o, b_v, b_o)


def bench(x, w_q, b_q, w_k, b_k, w_v, b_v, w_o, b_o):
    """Run with NTFF tracing; returns (output, exec_time_ns)."""
    nc = _get_nc()
    maps = _in_maps(x, w_q, b_q, w_k, b_k, w_v, b_v, w_o, b_o)
    res = run_bass_kernel_spmd(nc, maps, core_ids=list(range(8)), trace=True)
    return _combine(res.results, w_o, b_v, b_o), res.exec_time_ns
